# revision 19
# baseline (speedup 1.0000x reference)
"""TRN2 Bass kernel for nn_CustomQLoRABigNet: 6 blocks x (3 QLoRA linears),
ReLU, residual, LayerNorm. Data-parallel over 8 NeuronCores (4096 rows each).

Strategy vs baseline:
- Weights are dequantized ONCE (not per chunk) with the LoRA rank-32 update
  folded in on-chip: W_eff = (q-8)*scale + lb@la, stored as bf16. bf16
  stationary operands enable Fast Weight Load on the PE (fp32r weights pay a
  non-overlapped LDWEIGHTS per matmul) and halve weight DMA/SBUF.
- Chunk 0 interleaves the fold pipeline (DMA -> DVE dequant -> PE delta
  matmul -> DVE add) with the compute matmul stream; folded weights are
  written to DRAM and streamed back for chunks 1-3.
- Epilogues are spread across engines: ReLU+bias on Act, residual-add on DVE
  (scalar_tensor_tensor), LN apply split DVE/Pool/Act. LN sum/sumsq use a
  single [2,512] PSUM accumulator via 2-column selector matmuls.
- Hidden state stays fp32r (12-bit mantissa); 4 h buffers allow prefetching
  the next chunk's rows while the current chunk computes.
"""

import sys

sys.path.insert(0, "/opt/trn_rl_repo")

import numpy as np
import ml_dtypes

import concourse.bass as bass
from concourse import bacc, mybir
import concourse.tile as tile
from concourse.bass_utils import run_bass_kernel_spmd

f32 = mybir.dt.float32
f32r = mybir.dt.float32r
i8 = mybir.dt.int8
bf16 = mybir.dt.bfloat16
f16 = mybir.dt.float16
AF = mybir.ActivationFunctionType
Alu = mybir.AluOpType

N_CORES = 8
DIM = 1024
KT = 8  # 1024 / 128 partition tiles
NL = 18
RANK = 32
GROUP = 16
BATCH = 32768
RPC = BATCH // N_CORES  # rows per core
CHUNK = 1024  # columns (rows of x) processed per weight pass
NT = 512  # matmul moving free dim (one PSUM bank)
EPS = 1e-5


def fp32r_round(a: np.ndarray) -> np.ndarray:
    """Round-to-nearest-even fp32 -> fp32r (low 12 mantissa bits cleared)."""
    u = np.ascontiguousarray(a, dtype=np.float32).view(np.uint32)
    low = u & np.uint32(0xFFF)
    base = u & ~np.uint32(0xFFF)
    lsb = (u >> np.uint32(12)) & np.uint32(1)
    up = (low > 0x800) | ((low == 0x800) & (lsb == 1))
    out = base + np.where(up, np.uint32(0x1000), np.uint32(0)).astype(np.uint32)
    return out.view(np.float32)


def build_kernel(rows_per_core: int = RPC, chunk: int = CHUNK, n_layers: int = NL):
    nc = bacc.Bacc()
    n_chunks = rows_per_core // chunk
    ntiles = chunk // NT
    n_blocks = n_layers // 3

    x_d = nc.declare_dram_parameter("x_t", [128, KT, rows_per_core], f16, False)
    wq_d = nc.declare_dram_parameter("wqc", [n_layers, 128, KT, DIM], i8, False)
    sr_d = nc.declare_dram_parameter("srep", [n_layers, 128, KT, DIM], f32, False)
    la_d = nc.declare_dram_parameter("la_t", [n_layers, RANK, KT, 128], f16, False)
    lb_d = nc.declare_dram_parameter("lb_t", [n_layers, RANK, DIM], f16, False)
    bi_d = nc.declare_dram_parameter("bias_pp", [128, n_layers, KT], f32, False)
    ga_d = nc.declare_dram_parameter("gamma_pp", [128, 5, KT], f32, False)
    be_d = nc.declare_dram_parameter("beta_pp", [128, 5, KT], f32, False)
    on_d = nc.declare_dram_parameter("ones", [128, 128], f16, False)
    se_d = nc.declare_dram_parameter("sel", [128, 4], f16, False)
    y_d = nc.declare_dram_parameter("y_t", [128, KT, rows_per_core], f16, True)

    with tile.TileContext(nc) as tc:
        with (
            tc.tile_pool(name="persist", bufs=1) as pp,
            tc.tile_pool(name="wts", bufs=2) as wp,
            tc.tile_pool(name="fold", bufs=2) as fp,
            tc.tile_pool(name="work", bufs=2) as sp,
            tc.tile_pool(name="ps", bufs=1, space="PSUM") as ps,
            tc.tile_pool(name="dws", bufs=1, space="DRAM") as dp,
        ):
            # ---- persistent small tiles ----
            bias_t = pp.tile([128, n_layers, KT], f32)
            nc.sync.dma_start(bias_t[:, :, :], bi_d[:, :, :])
            gamma_t = pp.tile([128, 5, KT], f32)
            nc.sync.dma_start(gamma_t[:, :, :], ga_d[:, :, :])
            beta_t = pp.tile([128, 5, KT], f32)
            nc.sync.dma_start(beta_t[:, :, :], be_d[:, :, :])
            ones_t = pp.tile([128, 128], f16)
            nc.sync.dma_start(ones_t[:, :], on_d[:, :])
            sel_t = pp.tile([128, 4], f16)
            nc.sync.dma_start(sel_t[:, :], se_d[:, :])

            # hidden-state buffers: h[c%2]=block in/out (A), h[(c+1)%2]=next-x
            # stage, h[2]=B, h[3]=C
            hbuf = [
                pp.tile([128, KT, chunk], f16, name=f"h{i}") for i in range(4)
            ]

            # folded-weight DRAM scratch, one tile per layer for per-layer deps
            weff = [
                dp.tile([128, KT, DIM], f16, name=f"weff{l}", tag=f"weff{l}")
                for l in range(n_layers)
            ]

            class FoldState:
                """Dequant+LoRA-fold pipeline for one layer, emitted in 16
                steps interleaved with the previous layer's matmul stream."""

                def __init__(self, l, w_t):
                    self.l = l
                    self.w_t = w_t
                    self.wf = None

                def _prep(self, kt):
                    l = self.l
                    wqt = fp.tile([128, DIM], i8, tag="wq")
                    nc.sync.dma_start(wqt[:, :], wq_d[l, :, kt, :])
                    srt = fp.tile([128, DIM], f32, tag="sr")
                    nc.sync.dma_start(srt[:, :], sr_d[l, :, kt, :])
                    wf = fp.tile([128, DIM], f32r, tag="wf", bufs=1)
                    nc.vector.tensor_mul(wf[:, :], wqt[:, :], srt[:, :])
                    return wf

                def start(self):
                    l = self.l
                    self.la_t = fp.tile([RANK, KT, 128], f16, tag="la", bufs=1)
                    nc.sync.dma_start(self.la_t[:, :, :], la_d[l, :, :, :])
                    self.lb_t = fp.tile([RANK, DIM], f16, tag="lb", bufs=1)
                    nc.sync.dma_start(self.lb_t[:, :], lb_d[l, :, :])
                    self.wf = self._prep(0)

                def step(self, h):
                    # h in 0..15: kt = h//2, half = h%2
                    kt, half = h // 2, h % 2
                    cols = bass.ts(half, NT)
                    dps = ps.tile([128, NT], f32, tag="delta", bufs=1)
                    nc.tensor.matmul(
                        dps[:, :],
                        lhsT=self.la_t[:, kt, :],
                        rhs=self.lb_t[:, cols],
                        start=True,
                        stop=True,
                    )
                    nc.vector.tensor_add(
                        self.w_t[:, kt, cols], self.wf[:, cols], dps[:, :]
                    )
                    if half == 1 and kt < KT - 1:
                        self.wf = self._prep(kt + 1)

            import collections as _c

            tailq = _c.deque()  # deferred LN-tail stages, one per group boundary

            def make_ln_stages(blk, A, cols, hsum, sqs, osl7, hsq7):
                """LN tail as 4 deferred stages so the PE parts interleave
                with the following matmul stream instead of stalling it."""
                st = {}

                def reduce_stage():
                    s1p = ps.tile([1, NT], f32, tag="s1", bufs=1)
                    nc.tensor.matmul(s1p[:, :], lhsT=ones_t[:, 0:1], rhs=hsum[:, :],
                                     start=True, stop=False)
                    nc.tensor.matmul(s1p[:, :], lhsT=ones_t[:, 0:1], rhs=osl7,
                                     start=False, stop=True)
                    s2p = ps.tile([1, NT], f32, tag="s2", bufs=1)
                    nc.tensor.matmul(s2p[:, :], lhsT=ones_t[:, 0:1], rhs=sqs[:, :],
                                     start=True, stop=False)
                    nc.tensor.matmul(s2p[:, :], lhsT=ones_t[:, 0:1], rhs=hsq7[:, :],
                                     start=False, stop=True)
                    st["s1p"], st["s2p"] = s1p, s2p

                def stats_stage():
                    m = sp.tile([1, NT], f32, tag="m", bufs=1)
                    nc.vector.tensor_scalar(m[:, :], st["s1p"][:, :], 1.0 / DIM,
                                            None, Alu.mult)
                    msq = sp.tile([1, NT], f32, tag="msq", bufs=1)
                    nc.vector.tensor_mul(msq[:, :], m[:, :], m[:, :])
                    q2 = sp.tile([1, NT], f32, tag="q2", bufs=1)
                    nc.vector.tensor_scalar(q2[:, :], st["s2p"][:, :], 1.0 / DIM,
                                            EPS, Alu.mult, Alu.add)
                    var = sp.tile([1, NT], f32, tag="var", bufs=1)
                    nc.vector.tensor_sub(var[:, :], q2[:, :], msq[:, :])
                    lnv = sp.tile([1, NT], f32, tag="lnv", bufs=1)
                    nc.scalar.activation(lnv[:, :], var[:, :], AF.Ln)
                    inv = sp.tile([1, NT], f16, tag="inv", bufs=1)
                    nc.scalar.activation(inv[:, :], lnv[:, :], AF.Exp, scale=-0.5)
                    mi = sp.tile([1, NT], f16, tag="mi", bufs=1)
                    nc.vector.tensor_mul(mi[:, :], m[:, :], inv[:, :])
                    st["inv"], st["mi"] = inv, mi

                def bc_stage():
                    ib_ps = ps.tile([128, NT], f32, tag="bc", bufs=2)
                    nc.tensor.matmul(ib_ps[:, :], lhsT=ones_t[0:1, :],
                                     rhs=st["inv"][:, :], start=True, stop=True)
                    mb_ps = ps.tile([128, NT], f32, tag="bc", bufs=2)
                    nc.tensor.matmul(mb_ps[:, :], lhsT=ones_t[0:1, :],
                                     rhs=st["mi"][:, :], start=True, stop=True)
                    ib_sb = sp.tile([128, NT], f16, tag="ib", bufs=1)
                    nc.scalar.copy(ib_sb[:, :], ib_ps[:, :])
                    mb_sb = sp.tile([128, NT], f16, tag="mb", bufs=1)
                    nc.scalar.copy(mb_sb[:, :], mb_ps[:, :])
                    st["ib"], st["mb"] = ib_sb, mb_sb

                def apply_stage():
                    for kt in range(KT):
                        asl = A[:, kt, cols]
                        nc.vector.tensor_mul(asl, asl, st["ib"][:, :])
                        nc.vector.tensor_sub(asl, asl, st["mb"][:, :])
                        nc.scalar.activation(
                            asl, asl, AF.Identity,
                            bias=beta_t[:, blk, kt : kt + 1],
                            scale=gamma_t[:, blk, kt : kt + 1],
                        )

                noop = lambda: None
                return [reduce_stage, stats_stage, noop, bc_stage, noop, apply_stage]

            def compute_layer(l, w_t, h_in, h_out, hook):
                blk, j = l // 3, l % 3
                ln_here = j == 2 and blk < n_blocks - 1
                hstep = 0
                for nt in range(ntiles):
                    cols = bass.ts(nt, NT)
                    hsum = sqs = osl7 = hsq7 = None
                    for ot in range(KT):
                        y = ps.tile([128, NT], f32, tag="y", bufs=3)
                        for kt in range(KT):
                            nc.tensor.matmul(
                                y[:, :],
                                lhsT=w_t[:, kt, bass.ts(ot, 128)],
                                rhs=h_in[:, kt, cols],
                                start=(kt == 0),
                                stop=(kt == KT - 1),
                            )
                        osl = h_out[:, ot, cols]
                        if j < 2:
                            nc.scalar.activation(
                                osl, y[:, :], AF.Relu, bias=bias_t[:, l, ot : ot + 1]
                            )
                        else:
                            # residual add: h_out is A holding the block input
                            nc.vector.scalar_tensor_tensor(
                                osl, y[:, :], bias_t[:, l, ot : ot + 1], osl,
                                Alu.add, Alu.add,
                            )
                        if ln_here:
                            if ot < KT - 1:
                                hsq = sp.tile([128, NT], f16, tag="hsq", bufs=2)
                                nc.scalar.activation(hsq[:, :], osl, AF.Square)
                                if ot == 0:
                                    hsum = sp.tile([128, NT], f16, tag="hsum", bufs=2)
                                    nc.vector.tensor_copy(hsum[:, :], osl)
                                    sqs = sp.tile([128, NT], f16, tag="sqs", bufs=2)
                                    nc.vector.tensor_copy(sqs[:, :], hsq[:, :])
                                else:
                                    nc.vector.tensor_add(hsum[:, :], hsum[:, :], osl)
                                    nc.vector.tensor_add(sqs[:, :], sqs[:, :], hsq[:, :])
                            else:
                                # last tile feeds the reduce matmuls directly;
                                # square on DVE right behind the epilogue op
                                hsq7 = sp.tile([128, NT], f16, tag="hsq", bufs=2)
                                nc.vector.tensor_mul(hsq7[:, :], osl, osl)
                                osl7 = osl
                        if tailq:
                            tailq.popleft()()
                        if hook is not None:
                            hook(hstep)
                        hstep += 1
                    if ln_here:
                        tailq.extend(
                            make_ln_stages(blk, h_out, cols, hsum, sqs, osl7, hsq7)
                        )

            # ---- initial x load + fold of layer 0 ----
            nc.sync.dma_start(hbuf[0][:, :, :], x_d[:, :, bass.ts(0, chunk)])

            for c in range(n_chunks):
                A = hbuf[c % 2]
                stage = hbuf[(c + 1) % 2]
                B = hbuf[2]
                C = hbuf[3]
                if c + 1 < n_chunks:
                    nc.sync.dma_start(stage[:, :, :], x_d[:, :, bass.ts(c + 1, chunk)])

                # weights for layer 0 of this chunk
                w_cur = wp.tile([128, KT, DIM], f16, tag="w")
                if c == 0:
                    f0 = FoldState(0, w_cur)
                    f0.start()
                    for h in range(2 * KT):
                        f0.step(h)
                else:
                    nc.sync.dma_start(w_cur[:, :, :], weff[0][:, :, :])

                for l in range(n_layers):
                    j = l % 3
                    h_in = A if j == 0 else (B if j == 1 else C)
                    h_out = B if j == 0 else (C if j == 1 else A)

                    hook = None
                    w_next = None
                    if l + 1 < n_layers:
                        w_next = wp.tile([128, KT, DIM], f16, tag="w")
                        if c == 0:
                            fst = FoldState(l + 1, w_next)
                            fst.start()
                            hook = fst.step
                        else:
                            nc.sync.dma_start(w_next[:, :, :], weff[l + 1][:, :, :])

                    compute_layer(l, w_cur, h_in, h_out, hook)
                    if c == 0:
                        nc.sync.dma_start(weff[l][:, :, :], w_cur[:, :, :])
                    w_cur = w_next

                nc.sync.dma_start(y_d[:, :, bass.ts(c, chunk)], A[:, :, :])

    nc.compile()
    return nc


def prep_inputs(x, wq, scales, bias, lora_a, lora_b, gamma, beta,
                rows_per_core=RPC, n_layers=NL):
    """Host-side pure layout prep; returns per-core input maps."""
    nl = n_layers
    wqc = (wq[:nl].transpose(0, 2, 1).astype(np.int8) - 8)  # [l, k, o] centered
    wqc = wqc.reshape(nl, KT, 128, DIM).transpose(0, 2, 1, 3).copy()  # [l,p,kt,o]

    G = scales[:nl].reshape(nl, DIM, 64)  # [l, o, group]
    p_idx = np.arange(128)[:, None] // GROUP  # [128,1]
    kt_idx = np.arange(KT)[None, :] * (128 // GROUP)  # [1,8]
    gidx = p_idx + kt_idx  # [128, 8] -> group row index
    srep = G.transpose(0, 2, 1)[:, gidx, :].astype(np.float32).copy()  # [l,128,8,o]

    la_t = lora_a[:nl].reshape(nl, RANK, KT, 128).astype(np.float16)  # [l,r,kt,k]
    lb_t = lora_b[:nl].transpose(0, 2, 1).astype(np.float16).copy()  # [l, r, o]

    bias_pp = bias[:nl].reshape(nl, KT, 128).transpose(2, 0, 1).astype(np.float32).copy()
    gamma_pp = gamma.reshape(5, KT, 128).transpose(2, 0, 1).astype(np.float32).copy()
    beta_pp = beta.reshape(5, KT, 128).transpose(2, 0, 1).astype(np.float32).copy()

    sel = np.zeros((128, 4), np.float16)
    sel[:, 0] = 1.0
    sel[:, 3] = 1.0

    shared = {
        "wqc": wqc, "srep": srep, "la_t": la_t, "lb_t": lb_t,
        "bias_pp": bias_pp, "gamma_pp": gamma_pp, "beta_pp": beta_pp,
        "ones": np.ones((128, 128), np.float16), "sel": sel,
    }
    in_maps = []
    for c in range(x.shape[0] // rows_per_core):
        xs = x[c * rows_per_core : (c + 1) * rows_per_core]  # [rows, 1024]
        x_t = np.ascontiguousarray(
            xs.T.reshape(KT, 128, rows_per_core).transpose(1, 0, 2)
        ).astype(np.float16)
        in_maps.append({"x_t": x_t, **shared})
    return in_maps


def unshard_output(results, rows_per_core=RPC):
    outs = []
    for r in results:
        y_t = np.asarray(r["y_t"]).astype(np.float32).reshape(128, KT, rows_per_core)
        outs.append(y_t.transpose(2, 1, 0).reshape(rows_per_core, DIM))
    return np.ascontiguousarray(np.concatenate(outs, axis=0), dtype=np.float32)


def kernel(x, wq, scales, bias, lora_a, lora_b, gamma, beta):
    x, wq, scales, bias, lora_a, lora_b, gamma, beta = (
        np.asarray(a) for a in (x, wq, scales, bias, lora_a, lora_b, gamma, beta)
    )
    nc = build_kernel()
    in_maps = prep_inputs(x, wq, scales, bias, lora_a, lora_b, gamma, beta)
    res = run_bass_kernel_spmd(nc, in_maps, list(range(N_CORES)))
    return unshard_output(res.results)


# revision 20
# speedup vs baseline: 1.0222x; 1.0222x over previous
"""TRN2 Bass kernel for nn_CustomQLoRABigNet: 6 blocks x (3 QLoRA linears),
ReLU, residual, LayerNorm. Data-parallel over 8 NeuronCores (4096 rows each).

Strategy vs baseline:
- Weights are dequantized ONCE (not per chunk) with the LoRA rank-32 update
  folded in on-chip: W_eff = (q-8)*scale + lb@la, stored as bf16. bf16
  stationary operands enable Fast Weight Load on the PE (fp32r weights pay a
  non-overlapped LDWEIGHTS per matmul) and halve weight DMA/SBUF.
- Chunk 0 interleaves the fold pipeline (DMA -> DVE dequant -> PE delta
  matmul -> DVE add) with the compute matmul stream; folded weights are
  written to DRAM and streamed back for chunks 1-3.
- Epilogues are spread across engines: ReLU+bias on Act, residual-add on DVE
  (scalar_tensor_tensor), LN apply split DVE/Pool/Act. LN sum/sumsq use a
  single [2,512] PSUM accumulator via 2-column selector matmuls.
- Hidden state stays fp32r (12-bit mantissa); 4 h buffers allow prefetching
  the next chunk's rows while the current chunk computes.
"""

import sys

sys.path.insert(0, "/opt/trn_rl_repo")

import numpy as np
import ml_dtypes

import concourse.bass as bass
from concourse import bacc, mybir
import concourse.tile as tile
from concourse.bass_utils import run_bass_kernel_spmd

f32 = mybir.dt.float32
f32r = mybir.dt.float32r
i8 = mybir.dt.int8
bf16 = mybir.dt.bfloat16
f16 = mybir.dt.float16
AF = mybir.ActivationFunctionType
Alu = mybir.AluOpType

N_CORES = 8
DIM = 1024
KT = 8  # 1024 / 128 partition tiles
NL = 18
RANK = 32
GROUP = 16
BATCH = 32768
RPC = BATCH // N_CORES  # rows per core
CHUNK = 1024  # columns (rows of x) processed per weight pass
NT = 512  # matmul moving free dim (one PSUM bank)
EPS = 1e-5


def fp32r_round(a: np.ndarray) -> np.ndarray:
    """Round-to-nearest-even fp32 -> fp32r (low 12 mantissa bits cleared)."""
    u = np.ascontiguousarray(a, dtype=np.float32).view(np.uint32)
    low = u & np.uint32(0xFFF)
    base = u & ~np.uint32(0xFFF)
    lsb = (u >> np.uint32(12)) & np.uint32(1)
    up = (low > 0x800) | ((low == 0x800) & (lsb == 1))
    out = base + np.where(up, np.uint32(0x1000), np.uint32(0)).astype(np.uint32)
    return out.view(np.float32)


def build_kernel(rows_per_core: int = RPC, chunk: int = CHUNK, n_layers: int = NL):
    nc = bacc.Bacc()
    n_chunks = rows_per_core // chunk
    ntiles = chunk // NT
    n_blocks = n_layers // 3

    x_d = nc.declare_dram_parameter("x_t", [128, KT, rows_per_core], f16, False)
    wq_d = nc.declare_dram_parameter("wqc", [n_layers, 128, KT, DIM], i8, False)
    sr_d = nc.declare_dram_parameter("srep", [n_layers, 128, KT, DIM], f32, False)
    la_d = nc.declare_dram_parameter("la_t", [n_layers, RANK, KT, 128], f16, False)
    lb_d = nc.declare_dram_parameter("lb_t", [n_layers, RANK, DIM], f16, False)
    bi_d = nc.declare_dram_parameter("bias_pp", [128, n_layers, KT], f32, False)
    ga_d = nc.declare_dram_parameter("gamma_pp", [128, 5, KT], f32, False)
    be_d = nc.declare_dram_parameter("beta_pp", [128, 5, KT], f32, False)
    on_d = nc.declare_dram_parameter("ones", [128, 128], f16, False)
    se_d = nc.declare_dram_parameter("sel", [128, 4], f16, False)
    y_d = nc.declare_dram_parameter("y_t", [128, KT, rows_per_core], f16, True)

    with tile.TileContext(nc) as tc:
        with (
            tc.tile_pool(name="persist", bufs=1) as pp,
            tc.tile_pool(name="wts", bufs=2) as wp,
            tc.tile_pool(name="fold", bufs=2) as fp,
            tc.tile_pool(name="work", bufs=2) as sp,
            tc.tile_pool(name="ps", bufs=1, space="PSUM") as ps,
            tc.tile_pool(name="dws", bufs=1, space="DRAM") as dp,
        ):
            # ---- persistent small tiles ----
            bias_t = pp.tile([128, n_layers, KT], f32)
            nc.sync.dma_start(bias_t[:, :, :], bi_d[:, :, :])
            gamma_t = pp.tile([128, 5, KT], f32)
            nc.sync.dma_start(gamma_t[:, :, :], ga_d[:, :, :])
            beta_t = pp.tile([128, 5, KT], f32)
            nc.sync.dma_start(beta_t[:, :, :], be_d[:, :, :])
            ones_t = pp.tile([128, 128], f16)
            nc.sync.dma_start(ones_t[:, :], on_d[:, :])
            sel_t = pp.tile([128, 4], f16)
            nc.sync.dma_start(sel_t[:, :], se_d[:, :])

            # hidden-state buffers: h[c%2]=block in/out (A), h[(c+1)%2]=next-x
            # stage, h[2]=B, h[3]=C
            hbuf = [
                pp.tile([128, KT, chunk], f16, name=f"h{i}") for i in range(4)
            ]

            # folded-weight DRAM scratch, one tile per layer for per-layer deps
            weff = [
                dp.tile([128, KT, DIM], f16, name=f"weff{l}", tag=f"weff{l}")
                for l in range(n_layers)
            ]

            class FoldState:
                """Dequant+LoRA-fold pipeline for one layer, emitted in 16
                steps interleaved with the previous layer's matmul stream."""

                def __init__(self, l, w_t):
                    self.l = l
                    self.w_t = w_t
                    self.wf = None

                def _prep(self, kt):
                    l = self.l
                    wqt = fp.tile([128, DIM], i8, tag="wq")
                    nc.sync.dma_start(wqt[:, :], wq_d[l, :, kt, :])
                    srt = fp.tile([128, DIM], f32, tag="sr")
                    nc.sync.dma_start(srt[:, :], sr_d[l, :, kt, :])
                    wf = fp.tile([128, DIM], f32r, tag="wf", bufs=1)
                    nc.vector.tensor_mul(wf[:, :], wqt[:, :], srt[:, :])
                    return wf

                def start(self):
                    l = self.l
                    self.la_t = fp.tile([RANK, KT, 128], f16, tag="la", bufs=1)
                    nc.sync.dma_start(self.la_t[:, :, :], la_d[l, :, :, :])
                    self.lb_t = fp.tile([RANK, DIM], f16, tag="lb", bufs=1)
                    nc.sync.dma_start(self.lb_t[:, :], lb_d[l, :, :])
                    self.wf = self._prep(0)

                def step(self, h):
                    # h in 0..15: kt = h//2, half = h%2
                    kt, half = h // 2, h % 2
                    cols = bass.ts(half, NT)
                    dps = ps.tile([128, NT], f32, tag="delta", bufs=1)
                    nc.tensor.matmul(
                        dps[:, :],
                        lhsT=self.la_t[:, kt, :],
                        rhs=self.lb_t[:, cols],
                        start=True,
                        stop=True,
                    )
                    nc.vector.tensor_add(
                        self.w_t[:, kt, cols], self.wf[:, cols], dps[:, :]
                    )
                    if half == 1 and kt < KT - 1:
                        self.wf = self._prep(kt + 1)

            import collections as _c

            tailq = _c.deque()  # deferred LN-tail stages, one per group boundary

            def make_ln_stages(blk, A, cols, hsum, sqs, osl7, hsq7):
                """LN tail as 4 deferred stages so the PE parts interleave
                with the following matmul stream instead of stalling it."""
                st = {}

                def reduce_stage():
                    s1p = ps.tile([1, NT], f32, tag="s1", bufs=1)
                    nc.tensor.matmul(s1p[:, :], lhsT=ones_t[:, 0:1], rhs=hsum[:, :],
                                     start=True, stop=False)
                    nc.tensor.matmul(s1p[:, :], lhsT=ones_t[:, 0:1], rhs=osl7,
                                     start=False, stop=True)
                    s2p = ps.tile([1, NT], f32, tag="s2", bufs=1)
                    nc.tensor.matmul(s2p[:, :], lhsT=ones_t[:, 0:1], rhs=sqs[:, :],
                                     start=True, stop=False)
                    nc.tensor.matmul(s2p[:, :], lhsT=ones_t[:, 0:1], rhs=hsq7[:, :],
                                     start=False, stop=True)
                    st["s1p"], st["s2p"] = s1p, s2p

                def stats_stage():
                    m = sp.tile([1, NT], f32, tag="m", bufs=1)
                    nc.vector.tensor_scalar(m[:, :], st["s1p"][:, :], 1.0 / DIM,
                                            None, Alu.mult)
                    msq = sp.tile([1, NT], f32, tag="msq", bufs=1)
                    nc.vector.tensor_mul(msq[:, :], m[:, :], m[:, :])
                    q2 = sp.tile([1, NT], f32, tag="q2", bufs=1)
                    nc.vector.tensor_scalar(q2[:, :], st["s2p"][:, :], 1.0 / DIM,
                                            EPS, Alu.mult, Alu.add)
                    var = sp.tile([1, NT], f32, tag="var", bufs=1)
                    nc.vector.tensor_sub(var[:, :], q2[:, :], msq[:, :])
                    lnv = sp.tile([1, NT], f32, tag="lnv", bufs=1)
                    nc.scalar.activation(lnv[:, :], var[:, :], AF.Ln)
                    inv = sp.tile([1, NT], f16, tag="inv", bufs=1)
                    nc.scalar.activation(inv[:, :], lnv[:, :], AF.Exp, scale=-0.5)
                    mi = sp.tile([1, NT], f16, tag="mi", bufs=1)
                    nc.vector.tensor_mul(mi[:, :], m[:, :], inv[:, :])
                    st["inv"], st["mi"] = inv, mi

                def bc_stage():
                    ib_ps = ps.tile([128, NT], f32, tag="bc", bufs=2)
                    nc.tensor.matmul(ib_ps[:, :], lhsT=ones_t[0:1, :],
                                     rhs=st["inv"][:, :], start=True, stop=True)
                    mb_ps = ps.tile([128, NT], f32, tag="bc", bufs=2)
                    nc.tensor.matmul(mb_ps[:, :], lhsT=ones_t[0:1, :],
                                     rhs=st["mi"][:, :], start=True, stop=True)
                    ib_sb = sp.tile([128, NT], f16, tag="ib", bufs=1)
                    nc.scalar.copy(ib_sb[:, :], ib_ps[:, :])
                    mb_sb = sp.tile([128, NT], f16, tag="mb", bufs=1)
                    nc.scalar.copy(mb_sb[:, :], mb_ps[:, :])
                    st["ib"], st["mb"] = ib_sb, mb_sb

                def apply_stage():
                    for kt in range(KT):
                        asl = A[:, kt, cols]
                        nc.vector.tensor_mul(asl, asl, st["ib"][:, :])
                        nc.vector.tensor_sub(asl, asl, st["mb"][:, :])
                        nc.scalar.activation(
                            asl, asl, AF.Identity,
                            bias=beta_t[:, blk, kt : kt + 1],
                            scale=gamma_t[:, blk, kt : kt + 1],
                        )

                noop = lambda: None
                return [noop, reduce_stage, stats_stage, noop, bc_stage, apply_stage]

            def compute_layer(l, w_t, h_in, h_out, hook):
                blk, j = l // 3, l % 3
                ln_here = j == 2 and blk < n_blocks - 1
                hstep = 0
                for nt in range(ntiles):
                    cols = bass.ts(nt, NT)
                    hsum = sqs = osl7 = hsq7 = None
                    for ot in range(KT):
                        y = ps.tile([128, NT], f32, tag="y", bufs=3)
                        for kt in range(KT):
                            nc.tensor.matmul(
                                y[:, :],
                                lhsT=w_t[:, kt, bass.ts(ot, 128)],
                                rhs=h_in[:, kt, cols],
                                start=(kt == 0),
                                stop=(kt == KT - 1),
                            )
                        osl = h_out[:, ot, cols]
                        if j < 2:
                            nc.scalar.activation(
                                osl, y[:, :], AF.Relu, bias=bias_t[:, l, ot : ot + 1]
                            )
                        else:
                            # residual add: h_out is A holding the block input
                            nc.vector.scalar_tensor_tensor(
                                osl, y[:, :], bias_t[:, l, ot : ot + 1], osl,
                                Alu.add, Alu.add,
                            )
                        if ln_here:
                            if ot < KT - 1:
                                hsq = sp.tile([128, NT], f16, tag="hsq", bufs=2)
                                nc.scalar.activation(hsq[:, :], osl, AF.Square)
                                if ot == 0:
                                    hsum = sp.tile([128, NT], f16, tag="hsum", bufs=2)
                                    nc.vector.tensor_copy(hsum[:, :], osl)
                                    sqs = sp.tile([128, NT], f16, tag="sqs", bufs=2)
                                    nc.vector.tensor_copy(sqs[:, :], hsq[:, :])
                                else:
                                    nc.vector.tensor_add(hsum[:, :], hsum[:, :], osl)
                                    nc.vector.tensor_add(sqs[:, :], sqs[:, :], hsq[:, :])
                            else:
                                # last tile feeds the reduce matmuls directly
                                hsq7 = sp.tile([128, NT], f16, tag="hsq", bufs=2)
                                nc.scalar.activation(hsq7[:, :], osl, AF.Square)
                                osl7 = osl
                        if tailq:
                            tailq.popleft()()
                        if hook is not None:
                            hook(hstep)
                        hstep += 1
                    if ln_here:
                        tailq.extend(
                            make_ln_stages(blk, h_out, cols, hsum, sqs, osl7, hsq7)
                        )

            # ---- initial x load + fold of layer 0 ----
            nc.sync.dma_start(hbuf[0][:, :, :], x_d[:, :, bass.ts(0, chunk)])

            for c in range(n_chunks):
                A = hbuf[c % 2]
                stage = hbuf[(c + 1) % 2]
                B = hbuf[2]
                C = hbuf[3]
                if c + 1 < n_chunks:
                    nc.sync.dma_start(stage[:, :, :], x_d[:, :, bass.ts(c + 1, chunk)])

                # weights for layer 0 of this chunk
                w_cur = wp.tile([128, KT, DIM], f16, tag="w")
                if c == 0:
                    f0 = FoldState(0, w_cur)
                    f0.start()
                    for h in range(2 * KT):
                        f0.step(h)
                else:
                    nc.sync.dma_start(w_cur[:, :, :], weff[0][:, :, :])

                for l in range(n_layers):
                    j = l % 3
                    h_in = A if j == 0 else (B if j == 1 else C)
                    h_out = B if j == 0 else (C if j == 1 else A)

                    hook = None
                    w_next = None
                    if l + 1 < n_layers:
                        w_next = wp.tile([128, KT, DIM], f16, tag="w")
                        if c == 0:
                            fst = FoldState(l + 1, w_next)
                            fst.start()
                            hook = fst.step
                        else:
                            nc.sync.dma_start(w_next[:, :, :], weff[l + 1][:, :, :])

                    compute_layer(l, w_cur, h_in, h_out, hook)
                    if c == 0:
                        nc.sync.dma_start(weff[l][:, :, :], w_cur[:, :, :])
                    w_cur = w_next

                nc.sync.dma_start(y_d[:, :, bass.ts(c, chunk)], A[:, :, :])

    nc.compile()
    return nc


def prep_inputs(x, wq, scales, bias, lora_a, lora_b, gamma, beta,
                rows_per_core=RPC, n_layers=NL):
    """Host-side pure layout prep; returns per-core input maps."""
    nl = n_layers
    wqc = (wq[:nl].transpose(0, 2, 1).astype(np.int8) - 8)  # [l, k, o] centered
    wqc = wqc.reshape(nl, KT, 128, DIM).transpose(0, 2, 1, 3).copy()  # [l,p,kt,o]

    G = scales[:nl].reshape(nl, DIM, 64)  # [l, o, group]
    p_idx = np.arange(128)[:, None] // GROUP  # [128,1]
    kt_idx = np.arange(KT)[None, :] * (128 // GROUP)  # [1,8]
    gidx = p_idx + kt_idx  # [128, 8] -> group row index
    srep = G.transpose(0, 2, 1)[:, gidx, :].astype(np.float32).copy()  # [l,128,8,o]

    la_t = lora_a[:nl].reshape(nl, RANK, KT, 128).astype(np.float16)  # [l,r,kt,k]
    lb_t = lora_b[:nl].transpose(0, 2, 1).astype(np.float16).copy()  # [l, r, o]

    bias_pp = bias[:nl].reshape(nl, KT, 128).transpose(2, 0, 1).astype(np.float32).copy()
    gamma_pp = gamma.reshape(5, KT, 128).transpose(2, 0, 1).astype(np.float32).copy()
    beta_pp = beta.reshape(5, KT, 128).transpose(2, 0, 1).astype(np.float32).copy()

    sel = np.zeros((128, 4), np.float16)
    sel[:, 0] = 1.0
    sel[:, 3] = 1.0

    shared = {
        "wqc": wqc, "srep": srep, "la_t": la_t, "lb_t": lb_t,
        "bias_pp": bias_pp, "gamma_pp": gamma_pp, "beta_pp": beta_pp,
        "ones": np.ones((128, 128), np.float16), "sel": sel,
    }
    in_maps = []
    for c in range(x.shape[0] // rows_per_core):
        xs = x[c * rows_per_core : (c + 1) * rows_per_core]  # [rows, 1024]
        x_t = np.ascontiguousarray(
            xs.T.reshape(KT, 128, rows_per_core).transpose(1, 0, 2)
        ).astype(np.float16)
        in_maps.append({"x_t": x_t, **shared})
    return in_maps


def unshard_output(results, rows_per_core=RPC):
    outs = []
    for r in results:
        y_t = np.asarray(r["y_t"]).astype(np.float32).reshape(128, KT, rows_per_core)
        outs.append(y_t.transpose(2, 1, 0).reshape(rows_per_core, DIM))
    return np.ascontiguousarray(np.concatenate(outs, axis=0), dtype=np.float32)


def kernel(x, wq, scales, bias, lora_a, lora_b, gamma, beta):
    x, wq, scales, bias, lora_a, lora_b, gamma, beta = (
        np.asarray(a) for a in (x, wq, scales, bias, lora_a, lora_b, gamma, beta)
    )
    nc = build_kernel()
    in_maps = prep_inputs(x, wq, scales, bias, lora_a, lora_b, gamma, beta)
    res = run_bass_kernel_spmd(nc, in_maps, list(range(N_CORES)))
    return unshard_output(res.results)
